# revision 6
# baseline (speedup 1.0000x reference)
"""MemoryAugmentedLayer kernel for 8 trn2 NeuronCores.

Data-parallel over batch B=32768 (4096 rows/core); the two einsum partial
sums ([M,K] and [M,V]) are all-reduced between the write and read phases.
All activations are kept feature-major ([feat, batch]) on chip so every
chained matmul uses the weight as the stationary operand; batch-major
copies are made (via PE transpose) only where the einsum / output needs
them.

Precision: read path (projections, read scores/logits, read_vec) runs in
float32r (full PE rate, ~1e-4 rel err). The write path after the write
scores (Wwr logits, exp, einsum operands) runs in bf16 — it only feeds
the memory update, which is a ~5% perturbation of the memories, so bf16
error there is attenuated by ~20x in the output.
"""

import numpy as np

import concourse.bacc as bacc
import concourse.mybir as mybir
import concourse.tile as tile
from concourse import masks
from concourse.bass_utils import run_bass_kernel_spmd

F32 = mybir.dt.float32
F32R = mybir.dt.float32r
BF16 = mybir.dt.bfloat16

B, D, M, K, V = 32768, 256, 1024, 128, 128
N_CORES = 8
B_LOC = B // N_CORES          # 4096 rows per core
CHUNK = 512                   # batch columns processed per chunk
NCH = B_LOC // CHUNK          # 8 chunks
NBT = CHUNK // 128            # 4 batch tiles of 128 per chunk
MT = M // 128                 # 8 tiles of the memory dim
DT = D // 128                 # 2 tiles of the input dim
INV_B = 1.0 / B


def build_nc():
    nc = bacc.Bacc("TRN2", target_bir_lowering=False, debug=False,
                   num_devices=N_CORES)

    x = nc.dram_tensor("x", [B_LOC, D], F32, kind="ExternalInput")
    Wk = nc.dram_tensor("Wk", [D, K], F32, kind="ExternalInput")
    Wv = nc.dram_tensor("Wv", [D, V], F32, kind="ExternalInput")
    Wq = nc.dram_tensor("Wq", [D, K], F32, kind="ExternalInput")
    bk = nc.dram_tensor("bk", [K, 1], F32, kind="ExternalInput")
    bv = nc.dram_tensor("bv", [V, 1], F32, kind="ExternalInput")
    bq = nc.dram_tensor("bq", [K, 1], F32, kind="ExternalInput")
    Wwr = nc.dram_tensor("Wwr", [M, M], F32, kind="ExternalInput")
    Wrd = nc.dram_tensor("Wrd", [M, M], F32, kind="ExternalInput")
    bwr = nc.dram_tensor("bwr", [M, 1], F32, kind="ExternalInput")
    brd = nc.dram_tensor("brd", [M, 1], F32, kind="ExternalInput")
    km = nc.dram_tensor("key_memory", [M, K], F32, kind="ExternalInput")
    vm = nc.dram_tensor("value_memory", [M, V], F32, kind="ExternalInput")
    y = nc.dram_tensor("y", [B_LOC, V], F32, kind="ExternalOutput")

    with tile.TileContext(nc) as tc:
        _emit(nc, tc, x, Wk, Wv, Wq, bk, bv, bq, Wwr, Wrd, bwr, brd, km, vm, y)
    nc.compile()
    return nc


def _emit(nc, tc, x, Wk, Wv, Wq, bk, bv, bq, Wwr, Wrd, bwr, brd, km, vm, y):
    AF = mybir.ActivationFunctionType
    ALU = mybir.AluOpType
    AX = mybir.AxisListType

    with (
        tc.tile_pool(name="resident", bufs=1) as rp,
        tc.tile_pool(name="stage", bufs=2) as stage,
        tc.tile_pool(name="stream", bufs=2) as sp,
        tc.tile_pool(name="stream1", bufs=1) as sp1,
        tc.tile_pool(name="ps_acc", bufs=1, space="PSUM") as ps_acc,
        tc.tile_pool(name="ps_mm", bufs=2, space="PSUM") as ps_mm,
        tc.tile_pool(name="ps_tr", bufs=1, space="PSUM") as ps_tr,
        tc.tile_pool(name="dram", bufs=1, space="DRAM") as dp,
    ):
        # ---------------- setup: identities, weights, biases -------------
        ident = rp.tile([128, 128], F32)
        masks.make_identity(nc, ident[:])
        ident_b = rp.tile([128, 128], BF16)
        nc.vector.tensor_copy(ident_b[:], ident[:])

        ones_f = rp.tile([128, 1], F32)
        nc.gpsimd.memset(ones_f[:], 1.0)
        ones_r = rp.tile([128, 1], F32R)
        nc.vector.tensor_copy(ones_r[:], ones_f[:])
        one1 = rp.tile([1, 1], F32)
        nc.gpsimd.memset(one1[:], 1.0)

        # projection weights as lhsT ([d,128] blocks), rounded to f32r
        projw_r = rp.tile([128, DT, 3, 128], F32R)
        for j, W in enumerate((Wk, Wv, Wq)):
            for dt in range(DT):
                wst = stage.tile([128, 128], F32, tag="wst")
                nc.sync.dma_start(wst[:], W[dt * 128:(dt + 1) * 128, :])
                nc.vector.tensor_copy(projw_r[:, dt, j, :], wst[:])

        bias_p = rp.tile([128, 3], F32)
        for j, b in enumerate((bk, bv, bq)):
            nc.sync.dma_start(bias_p[:, j:j + 1], b[:])
        bias_wr = rp.tile([128, MT], F32)
        bias_rd = rp.tile([128, MT], F32)
        for mp in range(MT):
            nc.sync.dma_start(bias_wr[:, mp:mp + 1], bwr[mp * 128:(mp + 1) * 128, :])
            nc.sync.dma_start(bias_rd[:, mp:mp + 1], brd[mp * 128:(mp + 1) * 128, :])

        # dense weights as lhsT tiles [128, M]; Wwr in bf16 (write path),
        # Wrd in f32r (read path)
        wwr_b = [rp.tile([128, M], BF16, name=f"wwr_b{i}") for i in range(MT)]
        wrd_r = [rp.tile([128, M], F32R, name=f"wrd_r{i}") for i in range(MT)]
        for mk in range(MT):
            wst2 = stage.tile([128, M], F32, tag="wst2")
            nc.sync.dma_start(wst2[:], Wwr[mk * 128:(mk + 1) * 128, :])
            nc.vector.tensor_copy(wwr_b[mk][:], wst2[:])
        for mk in range(MT):
            wst3 = stage.tile([128, M], F32, tag="wst2")
            nc.sync.dma_start(wst3[:], Wrd[mk * 128:(mk + 1) * 128, :])
            nc.vector.tensor_copy(wrd_r[mk][:], wst3[:])

        # memories transposed: kmT ([k, M], f32r), vmT ([v, M], f32)
        kmT_r = rp.tile([128, M], F32R)
        vmT_f = rp.tile([128, M], F32)
        for mk in range(MT):
            mst = stage.tile([128, 128], F32, tag="mst")
            nc.sync.dma_start(mst[:], km[mk * 128:(mk + 1) * 128, :])
            ptr = ps_tr.tile([128, 128], F32, tag="tr")
            nc.tensor.matmul(ptr[:], mst[:], ident[:], is_transpose=True,
                             start=True, stop=True)
            nc.scalar.copy(kmT_r[:, mk * 128:(mk + 1) * 128], ptr[:])
            mst2 = stage.tile([128, 128], F32, tag="mst")
            nc.sync.dma_start(mst2[:], vm[mk * 128:(mk + 1) * 128, :])
            ptr2 = ps_tr.tile([128, 128], F32, tag="tr")
            nc.tensor.matmul(ptr2[:], mst2[:], ident[:], is_transpose=True,
                             start=True, stop=True)
            nc.vector.tensor_copy(vmT_f[:, mk * 128:(mk + 1) * 128], ptr2[:])

        # qry kept for phase 2
        qryT_r = rp.tile([128, B_LOC], F32R)

        # einsum partial accumulators, PSUM-resident across phase 1
        pk_lo = ps_acc.tile([128, 512], F32, tag="slot_a")
        pk_hi = ps_acc.tile([128, 512], F32, tag="slot_b")
        pv_lo = ps_acc.tile([128, 512], F32, tag="slot_c")
        pv_hi = ps_acc.tile([128, 512], F32, tag="slot_d")

        # ======================= PHASE 1 =================================
        for h in range(NCH):
            # ---- load + transpose x chunk -> xTr [128, dtile, CHUNK] f32r
            xTr = sp.tile([128, DT, CHUNK], F32R)
            for t in range(NBT):
                r0 = h * CHUNK + t * 128
                xa = sp.tile([128, D], F32, tag="xa", bufs=3)
                nc.sync.dma_start(xa[:], x[r0:r0 + 128, :])
                for dt in range(DT):
                    ptx = ps_tr.tile([128, 128], F32, tag="tr")
                    nc.tensor.matmul(ptx[:], xa[:, dt * 128:(dt + 1) * 128],
                                     ident[:], is_transpose=True,
                                     start=True, stop=True)
                    nc.any.tensor_copy(
                        xTr[:, dt, t * 128:(t + 1) * 128], ptx[:])

            # ---- projections + elu -> kvT/vvT/qryT (f32r)
            kvT = sp.tile([128, CHUNK], F32R)
            vvT = sp.tile([128, CHUNK], F32R)
            for j in range(3):
                pp = ps_mm.tile([128, CHUNK], F32, tag="mm")
                for dt in range(DT):
                    nc.tensor.matmul(pp[:], projw_r[:, dt, j, :], xTr[:, dt, :],
                                     start=(dt == 0), stop=(dt == DT - 1))
                bcol = bias_p[:, j:j + 1]
                tmin = sp.tile([128, CHUNK], F32, tag="tmin", bufs=2)
                nc.vector.tensor_scalar(out=tmin[:], in0=pp[:], scalar1=bcol,
                                        scalar2=0.0, op0=ALU.add, op1=ALU.min)
                texp = sp.tile([128, CHUNK], F32, tag="texp", bufs=2)
                nc.scalar.activation(texp[:], tmin[:], AF.Exp)
                trelu = sp.tile([128, CHUNK], F32, tag="trelu", bufs=2)
                nc.vector.tensor_scalar(out=trelu[:], in0=pp[:], scalar1=bcol,
                                        scalar2=0.0, op0=ALU.add, op1=ALU.max)
                dst = (kvT[:], vvT[:],
                       qryT_r[:, h * CHUNK:(h + 1) * CHUNK])[j]
                nc.vector.scalar_tensor_tensor(dst, texp[:], -1.0, trelu[:],
                                               ALU.add, ALU.add)

            # ---- write scores: wsT (bf16) [128, MT, CHUNK]
            wsT = sp1.tile([128, MT, CHUNK], BF16, tag="scoresT")
            for mt in range(MT):
                pws = ps_mm.tile([128, CHUNK], F32, tag="mm")
                nc.tensor.matmul(pws[:], kmT_r[:, mt * 128:(mt + 1) * 128],
                                 kvT[:], start=True, stop=True)
                nc.scalar.copy(wsT[:, mt, :], pws[:])

            # ---- write logits + exp + transpose to batch-major (bf16)
            expw_bm = sp1.tile([128, NBT, M], BF16)
            for mp in range(MT):
                pwl = ps_mm.tile([128, CHUNK], F32, tag="mm")
                for mk in range(MT):
                    nc.tensor.matmul(pwl[:],
                                     wwr_b[mk][:, mp * 128:(mp + 1) * 128],
                                     wsT[:, mk, :],
                                     start=(mk == 0), stop=(mk == MT - 1))
                eT = sp.tile([128, CHUNK], BF16, tag="eT", bufs=2)
                nc.scalar.activation(eT[:], pwl[:], AF.Exp,
                                     bias=bias_wr[:, mp:mp + 1])
                for t in range(NBT):
                    pte = ps_tr.tile([128, 128], BF16, tag="trb")
                    nc.tensor.matmul(pte[:], eT[:, t * 128:(t + 1) * 128],
                                     ident_b[:], is_transpose=True,
                                     start=True, stop=True)
                    nc.any.tensor_copy(
                        expw_bm[:, t, mp * 128:(mp + 1) * 128], pte[:])

            # ---- softmax denominators (per batch row) + scaled kv/vv (bm)
            rw = sp.tile([128, NBT], F32, tag="rw")
            sw = sp.tile([128, NBT], F32, tag="sw")
            for t in range(NBT):
                nc.vector.tensor_reduce(sw[:, t:t + 1], expw_bm[:, t, :],
                                        AX.X, ALU.add)
            nc.vector.reciprocal(rw[:], sw[:])

            kv_sc = sp.tile([128, NBT, 128], BF16, tag="kv_sc")
            vv_sc = sp.tile([128, NBT, 128], BF16, tag="vv_sc")
            for t in range(NBT):
                ptk = ps_tr.tile([128, 128], F32, tag="tr")
                nc.tensor.matmul(ptk[:],
                                 kvT[:, t * 128:(t + 1) * 128].bitcast(F32),
                                 ident[:], is_transpose=True,
                                 start=True, stop=True)
                nc.vector.tensor_scalar_mul(kv_sc[:, t, :], ptk[:],
                                            rw[:, t:t + 1])
                ptv = ps_tr.tile([128, 128], F32, tag="tr")
                nc.tensor.matmul(ptv[:],
                                 vvT[:, t * 128:(t + 1) * 128].bitcast(F32),
                                 ident[:], is_transpose=True,
                                 start=True, stop=True)
                nc.vector.tensor_scalar_mul(vv_sc[:, t, :], ptv[:],
                                            rw[:, t:t + 1])

            # ---- einsum partials, accumulated in PSUM across all chunks
            for t in range(NBT):
                f = h == 0 and t == 0
                l = h == NCH - 1 and t == NBT - 1
                nc.tensor.matmul(pk_lo[:], kv_sc[:, t, :], expw_bm[:, t, 0:512],
                                 start=f, stop=l, skip_group_check=True)
                nc.tensor.matmul(pk_hi[:], kv_sc[:, t, :], expw_bm[:, t, 512:M],
                                 start=f, stop=l, skip_group_check=True)
                nc.tensor.matmul(pv_lo[:], vv_sc[:, t, :], expw_bm[:, t, 0:512],
                                 start=f, stop=l, skip_group_check=True)
                nc.tensor.matmul(pv_hi[:], vv_sc[:, t, :], expw_bm[:, t, 512:M],
                                 start=f, stop=l, skip_group_check=True)

        # ================== ALLREDUCE of partials ========================
        part_sb = rp.tile([128, 2 * M], F32)
        nc.vector.tensor_copy(part_sb[:, 0:512], pk_lo[:])
        nc.vector.tensor_copy(part_sb[:, 512:1024], pk_hi[:])
        nc.vector.tensor_copy(part_sb[:, 1024:1536], pv_lo[:])
        nc.vector.tensor_copy(part_sb[:, 1536:2048], pv_hi[:])
        cc_in = dp.tile([128, 2 * M], F32)
        cc_out = dp.tile([128, 2 * M], F32)
        nc.sync.dma_start(cc_in[:], part_sb[:])
        nc.gpsimd.collective_compute(
            "AllReduce", mybir.AluOpType.add,
            replica_groups=[list(range(N_CORES))],
            ins=[cc_in.opt()], outs=[cc_out.opt()],
        )
        red_sb = rp.tile([128, 2 * M], F32)
        nc.sync.dma_start(red_sb[:], cc_out[:])

        # ---- memory update: km_newT (f32r), vm_new ([m,v] blocks, f32r)
        km_newT_r = rp.tile([128, M], F32R)
        nc.vector.scalar_tensor_tensor(km_newT_r[:], red_sb[:, 0:M], INV_B,
                                       kmT_r[:].bitcast(F32), ALU.mult, ALU.add)
        vm_newT_f = rp.tile([128, M], F32)
        nc.vector.scalar_tensor_tensor(vm_newT_f[:], red_sb[:, M:2 * M], INV_B,
                                       vmT_f[:], ALU.mult, ALU.add)
        vmn_r = rp.tile([128, MT, 128], F32R)
        for mk in range(MT):
            ptm = ps_tr.tile([128, 128], F32, tag="tr")
            nc.tensor.matmul(ptm[:], vm_newT_f[:, mk * 128:(mk + 1) * 128],
                             ident[:], is_transpose=True, start=True, stop=True)
            nc.vector.tensor_copy(vmn_r[:, mk, :], ptm[:])

        # ======================= PHASE 2 =================================
        for h in range(NCH):
            qslice = qryT_r[:, h * CHUNK:(h + 1) * CHUNK]

            rsT = sp1.tile([128, MT, CHUNK], F32R, tag="scoresT")
            for mt in range(MT):
                prs = ps_mm.tile([128, CHUNK], F32, tag="mm")
                nc.tensor.matmul(prs[:], km_newT_r[:, mt * 128:(mt + 1) * 128],
                                 qslice, start=True, stop=True)
                nc.scalar.copy(rsT[:, mt, :], prs[:])

            u_ps = ps_acc.tile([128, CHUNK], F32, tag="slot_a")
            s_ps = ps_acc.tile([1, CHUNK], F32, tag="slot_b")
            for mp in range(MT):
                prl = ps_mm.tile([128, CHUNK], F32, tag="mm")
                for mk in range(MT):
                    nc.tensor.matmul(prl[:],
                                     wrd_r[mk][:, mp * 128:(mp + 1) * 128],
                                     rsT[:, mk, :],
                                     start=(mk == 0), stop=(mk == MT - 1))
                erT = sp.tile([128, CHUNK], F32R, tag="erT", bufs=2)
                nc.scalar.activation(erT[:], prl[:], AF.Exp,
                                     bias=bias_rd[:, mp:mp + 1])
                nc.tensor.matmul(u_ps[:], vmn_r[:, mp, :], erT[:],
                                 start=(mp == 0), stop=(mp == MT - 1),
                                 skip_group_check=True)
                nc.tensor.matmul(s_ps[:], ones_r[:], erT[:],
                                 start=(mp == 0), stop=(mp == MT - 1),
                                 skip_group_check=True)

            # transpose denominators [1, CHUNK] -> [128, NBT] and invert
            s_sb = sp.tile([1, CHUNK], F32, tag="s_sb")
            nc.scalar.copy(s_sb[:], s_ps[:])
            s_cols = sp.tile([128, NBT], F32, tag="s_cols")
            for t in range(NBT):
                pst = ps_acc.tile([128, 1], F32, tag="slot_c")
                nc.tensor.matmul(pst[:], s_sb[0:1, t * 128:(t + 1) * 128],
                                 one1[:], start=True, stop=True)
                nc.vector.tensor_copy(s_cols[:, t:t + 1], pst[:])
            r_cols = sp.tile([128, NBT], F32, tag="r_cols")
            nc.vector.reciprocal(r_cols[:], s_cols[:])

            # read_vec: transpose u back to batch-major, scale, store
            u_sb = sp.tile([128, CHUNK], F32, tag="u_sb")
            nc.scalar.copy(u_sb[:], u_ps[:])
            for t in range(NBT):
                ptu = ps_tr.tile([128, 128], F32, tag="tr")
                nc.tensor.matmul(ptu[:], u_sb[:, t * 128:(t + 1) * 128],
                                 ident[:], is_transpose=True,
                                 start=True, stop=True)
                ot = sp.tile([128, V], F32, tag="ot", bufs=3)
                nc.vector.tensor_scalar_mul(ot[:], ptu[:], r_cols[:, t:t + 1])
                r0 = h * CHUNK + t * 128
                nc.sync.dma_start(y[r0:r0 + 128, :], ot[:])


_NC_CACHE = None


def _get_nc():
    global _NC_CACHE
    if _NC_CACHE is None:
        _NC_CACHE = build_nc()
    return _NC_CACHE


def kernel(**inputs):
    nc = _get_nc()
    xs = np.ascontiguousarray(np.asarray(inputs["x"], dtype=np.float32))
    rep = {}
    for name in ("Wk", "Wv", "Wq", "Wwr", "Wrd", "key_memory", "value_memory"):
        rep[name] = np.ascontiguousarray(np.asarray(inputs[name], np.float32))
    for name in ("bk", "bv", "bq", "bwr", "brd"):
        rep[name] = np.ascontiguousarray(
            np.asarray(inputs[name], np.float32).reshape(-1, 1))
    in_maps = []
    for c in range(N_CORES):
        m = {"x": xs[c * B_LOC:(c + 1) * B_LOC]}
        m.update(rep)
        in_maps.append(m)
    res = run_bass_kernel_spmd(nc, in_maps, core_ids=list(range(N_CORES)))
    return np.concatenate([r["y"] for r in res.results], axis=0)


# revision 9
# speedup vs baseline: 1.0480x; 1.0480x over previous
"""MemoryAugmentedLayer kernel for 8 trn2 NeuronCores.

Data-parallel over batch B=32768 (4096 rows/core); the two einsum partial
sums ([M,K] and [M,V]) are all-reduced between the write and read phases.

Structure (per core):
- Associativity: write logits = key_vec @ G with G = key_memory.T @ Wwr
  precomputed once (and read logits = qry_vec @ H with H = km_new.T @ Wrd
  computed once after the all-reduce). This removes the [B,M] score
  intermediates entirely and keeps every streaming matmul a 128-deep
  contraction with the weight stationary.
- Activations stay feature-major ([feat, batch]) on chip; PE transposes
  produce the batch-major copies the einsum and the output need.
- Precision: read path in float32r (~1e-4 rel err, full PE rate); write
  path (G, exp weights, einsum operands) in bf16 — it only perturbs the
  memory update, which is a ~5% correction to the memories.
"""

import numpy as np

import concourse.bacc as bacc
import concourse.mybir as mybir
import concourse.tile as tile
from concourse import masks
from concourse.bass_utils import run_bass_kernel_spmd

F32 = mybir.dt.float32
F32R = mybir.dt.float32r
BF16 = mybir.dt.bfloat16

B, D, M, K, V = 32768, 256, 1024, 128, 128
N_CORES = 8
B_LOC = B // N_CORES          # 4096 rows per core
CHUNK = 512                   # batch columns processed per chunk
NCH = B_LOC // CHUNK          # 8 chunks
NBT = CHUNK // 128            # 4 batch tiles of 128 per chunk
MT = M // 128                 # 8 tiles of the memory dim
DT = D // 128                 # 2 tiles of the input dim
INV_B = 1.0 / B


def build_nc(repeat=1):
    nc = bacc.Bacc("TRN2", target_bir_lowering=False, debug=False,
                   num_devices=N_CORES)

    x = nc.dram_tensor("x", [B_LOC, D], F32, kind="ExternalInput")
    Wk = nc.dram_tensor("Wk", [D, K], F32, kind="ExternalInput")
    Wv = nc.dram_tensor("Wv", [D, V], F32, kind="ExternalInput")
    Wq = nc.dram_tensor("Wq", [D, K], F32, kind="ExternalInput")
    bk = nc.dram_tensor("bk", [K, 1], F32, kind="ExternalInput")
    bv = nc.dram_tensor("bv", [V, 1], F32, kind="ExternalInput")
    bq = nc.dram_tensor("bq", [K, 1], F32, kind="ExternalInput")
    Wwr = nc.dram_tensor("Wwr", [M, M], F32, kind="ExternalInput")
    Wrd = nc.dram_tensor("Wrd", [M, M], F32, kind="ExternalInput")
    bwr = nc.dram_tensor("bwr", [M, 1], F32, kind="ExternalInput")
    brd = nc.dram_tensor("brd", [M, 1], F32, kind="ExternalInput")
    km = nc.dram_tensor("key_memory", [M, K], F32, kind="ExternalInput")
    vm = nc.dram_tensor("value_memory", [M, V], F32, kind="ExternalInput")
    y = nc.dram_tensor("y", [B_LOC, V], F32, kind="ExternalOutput")

    with tile.TileContext(nc) as tc:
        _emit(nc, tc, x, Wk, Wv, Wq, bk, bv, bq, Wwr, Wrd, bwr, brd, km, vm, y,
              repeat=repeat)
    nc.compile()
    return nc


def _emit(nc, tc, x, Wk, Wv, Wq, bk, bv, bq, Wwr, Wrd, bwr, brd, km, vm, y,
          repeat=1):
    AF = mybir.ActivationFunctionType
    ALU = mybir.AluOpType

    with (
        tc.tile_pool(name="resident", bufs=1) as rp,
        tc.tile_pool(name="stage", bufs=2) as stage,
        tc.tile_pool(name="stream", bufs=2) as sp,
        tc.tile_pool(name="stream1", bufs=1) as sp1,
        tc.tile_pool(name="ps_acc", bufs=1, space="PSUM") as ps_acc,
        tc.tile_pool(name="ps_mm", bufs=2, space="PSUM") as ps_mm,
        tc.tile_pool(name="ps_tr", bufs=1, space="PSUM") as ps_tr,
        tc.tile_pool(name="dram", bufs=1, space="DRAM") as dp,
    ):
        # ---------------- setup: identities, ones, biases ----------------
        ident = rp.tile([128, 128], F32)
        masks.make_identity(nc, ident[:])
        ident_b = rp.tile([128, 128], BF16)
        nc.vector.tensor_copy(ident_b[:], ident[:])

        ones_f = rp.tile([128, 1], F32)
        nc.gpsimd.memset(ones_f[:], 1.0)
        ones_r = rp.tile([128, 1], F32R)
        nc.vector.tensor_copy(ones_r[:], ones_f[:])
        one1 = rp.tile([1, 1], F32)
        nc.gpsimd.memset(one1[:], 1.0)

        # projection weights as lhsT ([d,128] blocks), rounded to f32r
        projw_r = rp.tile([128, DT, 3, 128], F32R)
        for j, W in enumerate((Wk, Wv, Wq)):
            for dt in range(DT):
                wst = stage.tile([128, 128], F32, tag="wst")
                nc.sync.dma_start(wst[:], W[dt * 128:(dt + 1) * 128, :])
                nc.vector.tensor_copy(projw_r[:, dt, j, :], wst[:])

        bias_p = rp.tile([128, 3], F32)
        for j, b in enumerate((bk, bv, bq)):
            nc.sync.dma_start(bias_p[:, j:j + 1], b[:])
        bias_wr = rp.tile([128, MT], F32)
        bias_rd = rp.tile([128, MT], F32)
        for mp in range(MT):
            nc.sync.dma_start(bias_wr[:, mp:mp + 1], bwr[mp * 128:(mp + 1) * 128, :])
            nc.sync.dma_start(bias_rd[:, mp:mp + 1], brd[mp * 128:(mp + 1) * 128, :])

        # ---- G = key_memory.T @ Wwr (bf16), kmT/vmT (transposed, f32) ----
        kmT_f = rp.tile([128, M], F32)
        vmT_f = rp.tile([128, M], F32)
        g_lo = ps_acc.tile([128, 512], F32, tag="slot_a")
        g_hi = ps_acc.tile([128, 512], F32, tag="slot_b")
        for mk in range(MT):
            mst = stage.tile([128, 128], F32, tag="mst")
            nc.sync.dma_start(mst[:], km[mk * 128:(mk + 1) * 128, :])
            km_b = stage.tile([128, 128], BF16, tag="km_b")
            nc.vector.tensor_copy(km_b[:], mst[:])
            wwrt = stage.tile([128, M], F32, tag="wbig")
            nc.sync.dma_start(wwrt[:], Wwr[mk * 128:(mk + 1) * 128, :])
            wwrt_b = stage.tile([128, M], BF16, tag="wbig_b")
            nc.vector.tensor_copy(wwrt_b[:], wwrt[:])
            nc.tensor.matmul(g_lo[:], km_b[:], wwrt_b[:, 0:512],
                             start=(mk == 0), stop=(mk == MT - 1),
                             skip_group_check=True)
            nc.tensor.matmul(g_hi[:], km_b[:], wwrt_b[:, 512:M],
                             start=(mk == 0), stop=(mk == MT - 1),
                             skip_group_check=True)
            ptr = ps_tr.tile([128, 128], F32, tag="tr")
            nc.tensor.matmul(ptr[:], mst[:], ident[:], is_transpose=True,
                             start=True, stop=True)
            nc.scalar.copy(kmT_f[:, mk * 128:(mk + 1) * 128], ptr[:])
            mst2 = stage.tile([128, 128], F32, tag="mst")
            nc.sync.dma_start(mst2[:], vm[mk * 128:(mk + 1) * 128, :])
            ptr2 = ps_tr.tile([128, 128], F32, tag="tr")
            nc.tensor.matmul(ptr2[:], mst2[:], ident[:], is_transpose=True,
                             start=True, stop=True)
            nc.scalar.copy(vmT_f[:, mk * 128:(mk + 1) * 128], ptr2[:])
        G_b = rp.tile([128, M], BF16)
        nc.vector.tensor_copy(G_b[:, 0:512], g_lo[:])
        nc.vector.tensor_copy(G_b[:, 512:M], g_hi[:])

        # Wrd resident as lhsT tiles [128, M] f32r (read path)
        wrd_r = [rp.tile([128, M], F32R, name=f"wrd_r{i}") for i in range(MT)]
        for mk in range(MT):
            wst3 = stage.tile([128, M], F32, tag="wbig")
            nc.sync.dma_start(wst3[:], Wrd[mk * 128:(mk + 1) * 128, :])
            nc.vector.tensor_copy(wrd_r[mk][:], wst3[:])

        # qry kept for phase 2
        qryT_r = rp.tile([128, B_LOC], F32R)

        for _rep in range(repeat):
            _emit_rep(nc, tc, x, y, rp, sp, sp1, ps_acc, ps_mm, ps_tr, dp,
                      ident, ident_b, ones_r, one1, projw_r, bias_p, bias_wr,
                      bias_rd, G_b, wrd_r, kmT_f, vmT_f, qryT_r)


def _emit_rep(nc, tc, x, y, rp, sp, sp1, ps_acc, ps_mm, ps_tr, dp,
              ident, ident_b, ones_r, one1, projw_r, bias_p, bias_wr,
              bias_rd, G_b, wrd_r, kmT_f, vmT_f, qryT_r):
    AF = mybir.ActivationFunctionType
    ALU = mybir.AluOpType
    AX = mybir.AxisListType

    # einsum partial accumulators, PSUM-resident across phase 1
    pk_lo = ps_acc.tile([128, 512], F32, tag="slot_a")
    pk_hi = ps_acc.tile([128, 512], F32, tag="slot_b")
    pv_lo = ps_acc.tile([128, 512], F32, tag="slot_c")
    pv_hi = ps_acc.tile([128, 512], F32, tag="slot_d")

    # ======================= PHASE 1 =====================================
    for h in range(NCH):
        # ---- load + transpose x chunk -> xTr [128, dtile, CHUNK] f32r
        xTr = sp.tile([128, DT, CHUNK], F32R, tag="xTr")
        for t in range(NBT):
            r0 = h * CHUNK + t * 128
            xa = sp.tile([128, D], F32, tag="xa", bufs=3)
            nc.sync.dma_start(xa[:], x[r0:r0 + 128, :])
            for dt in range(DT):
                ptx = ps_tr.tile([128, 128], F32, tag="tr")
                nc.tensor.matmul(ptx[:], xa[:, dt * 128:(dt + 1) * 128],
                                 ident[:], is_transpose=True,
                                 start=True, stop=True)
                nc.any.tensor_copy(xTr[:, dt, t * 128:(t + 1) * 128], ptx[:])

        # ---- projections + elu -> kvT/vvT (bf16), qryT (f32r)
        kvT = sp.tile([128, CHUNK], BF16, tag="kvT")
        vvT = sp.tile([128, CHUNK], BF16, tag="vvT")
        for j in range(3):
            pp = ps_mm.tile([128, CHUNK], F32, tag="mm")
            for dt in range(DT):
                nc.tensor.matmul(pp[:], projw_r[:, dt, j, :], xTr[:, dt, :],
                                 start=(dt == 0), stop=(dt == DT - 1))
            bcol = bias_p[:, j:j + 1]
            tmin = sp.tile([128, CHUNK], F32, tag="tmin", bufs=2)
            nc.vector.tensor_scalar(out=tmin[:], in0=pp[:], scalar1=bcol,
                                    scalar2=0.0, op0=ALU.add, op1=ALU.min)
            texp = sp.tile([128, CHUNK], F32, tag="texp", bufs=2)
            nc.scalar.activation(texp[:], tmin[:], AF.Exp)
            trelu = sp.tile([128, CHUNK], F32, tag="trelu", bufs=2)
            nc.vector.tensor_scalar(out=trelu[:], in0=pp[:], scalar1=bcol,
                                    scalar2=0.0, op0=ALU.add, op1=ALU.max)
            dst = (kvT[:], vvT[:],
                   qryT_r[:, h * CHUNK:(h + 1) * CHUNK])[j]
            nc.vector.scalar_tensor_tensor(dst, texp[:], -1.0, trelu[:],
                                           ALU.add, ALU.add)

        # ---- write logits (via G) + exp + batched transpose to batch-major
        expw_bm = sp1.tile([128, NBT, M], BF16, tag="expw_bm")
        for mp in range(MT):
            pwl = ps_mm.tile([128, CHUNK], F32, tag="mm")
            nc.tensor.matmul(pwl[:], G_b[:, mp * 128:(mp + 1) * 128], kvT[:],
                             start=True, stop=True)
            eT = sp.tile([128, CHUNK], BF16, tag="eT", bufs=2)
            nc.scalar.activation(eT[:], pwl[:], AF.Exp,
                                 bias=bias_wr[:, mp:mp + 1])
            ptb = ps_tr.tile([128, NBT, 128], BF16, tag="trb")
            for t in range(NBT):
                nc.tensor.matmul(ptb[:, t, :], eT[:, t * 128:(t + 1) * 128],
                                 ident_b[:], is_transpose=True,
                                 start=True, stop=True, skip_group_check=True)
            nc.any.tensor_copy(expw_bm[:, :, mp * 128:(mp + 1) * 128], ptb[:])

        # ---- softmax denominators (per batch row) + scaled kv/vv (bm)
        rw = sp.tile([128, NBT], F32, tag="rw")
        sw = sp.tile([128, NBT], F32, tag="sw")
        for t in range(NBT):
            nc.vector.tensor_reduce(sw[:, t:t + 1], expw_bm[:, t, :],
                                    AX.X, ALU.add)
        nc.vector.reciprocal(rw[:], sw[:])

        kv_sc = sp.tile([128, NBT, 128], BF16, tag="kv_sc")
        vv_sc = sp.tile([128, NBT, 128], BF16, tag="vv_sc")
        for src, dstt in ((kvT, kv_sc), (vvT, vv_sc)):
            ptk = ps_tr.tile([128, NBT, 128], BF16, tag="trb")
            for t in range(NBT):
                nc.tensor.matmul(ptk[:, t, :], src[:, t * 128:(t + 1) * 128],
                                 ident_b[:], is_transpose=True,
                                 start=True, stop=True, skip_group_check=True)
            for t in range(NBT):
                nc.vector.tensor_scalar_mul(dstt[:, t, :], ptk[:, t, :],
                                            rw[:, t:t + 1])

        # ---- einsum partials, accumulated in PSUM across all chunks
        for t in range(NBT):
            f = h == 0 and t == 0
            l = h == NCH - 1 and t == NBT - 1
            nc.tensor.matmul(pk_lo[:], kv_sc[:, t, :], expw_bm[:, t, 0:512],
                             start=f, stop=l, skip_group_check=True)
            nc.tensor.matmul(pk_hi[:], kv_sc[:, t, :], expw_bm[:, t, 512:M],
                             start=f, stop=l, skip_group_check=True)
            nc.tensor.matmul(pv_lo[:], vv_sc[:, t, :], expw_bm[:, t, 0:512],
                             start=f, stop=l, skip_group_check=True)
            nc.tensor.matmul(pv_hi[:], vv_sc[:, t, :], expw_bm[:, t, 512:M],
                             start=f, stop=l, skip_group_check=True)

    # ================== ALLREDUCE of partials ============================
    part_sb = rp.tile([128, 2 * M], F32, tag="part_sb")
    nc.vector.tensor_copy(part_sb[:, 0:512], pk_lo[:])
    nc.vector.tensor_copy(part_sb[:, 512:1024], pk_hi[:])
    nc.vector.tensor_copy(part_sb[:, 1024:1536], pv_lo[:])
    nc.vector.tensor_copy(part_sb[:, 1536:2048], pv_hi[:])
    cc_in = dp.tile([128, 2 * M], F32, tag="cc_in")
    cc_out = dp.tile([128, 2 * M], F32, tag="cc_out")
    nc.sync.dma_start(cc_in[:], part_sb[:])
    nc.gpsimd.collective_compute(
        "AllReduce", mybir.AluOpType.add,
        replica_groups=[list(range(N_CORES))],
        ins=[cc_in.opt()], outs=[cc_out.opt()],
    )
    red_sb = rp.tile([128, 2 * M], F32, tag="red_sb")
    nc.sync.dma_start(red_sb[:], cc_out[:])

    # ---- memory update + H = km_new.T @ Wrd (f32r) ----------------------
    km_newT = rp.tile([128, M], F32, tag="km_newT")
    nc.vector.scalar_tensor_tensor(km_newT[:], red_sb[:, 0:M], INV_B,
                                   kmT_f[:], ALU.mult, ALU.add)
    vm_newT = rp.tile([128, M], F32, tag="vm_newT")
    nc.vector.scalar_tensor_tensor(vm_newT[:], red_sb[:, M:2 * M], INV_B,
                                   vmT_f[:], ALU.mult, ALU.add)
    # transpose km_new/vm_new back to [m, *] blocks (f32r)
    kmn_mk = rp.tile([128, MT, 128], F32R, tag="kmn_mk")
    vmn_r = rp.tile([128, MT, 128], F32R, tag="vmn_r")
    for mk in range(MT):
        ptm = ps_tr.tile([128, 128], F32, tag="tr")
        nc.tensor.matmul(ptm[:], km_newT[:, mk * 128:(mk + 1) * 128],
                         ident[:], is_transpose=True, start=True, stop=True)
        nc.any.tensor_copy(kmn_mk[:, mk, :], ptm[:])
        ptm2 = ps_tr.tile([128, 128], F32, tag="tr")
        nc.tensor.matmul(ptm2[:], vm_newT[:, mk * 128:(mk + 1) * 128],
                         ident[:], is_transpose=True, start=True, stop=True)
        nc.any.tensor_copy(vmn_r[:, mk, :], ptm2[:])
    h_lo = ps_acc.tile([128, 512], F32, tag="slot_a")
    h_hi = ps_acc.tile([128, 512], F32, tag="slot_b")
    for mk in range(MT):
        nc.tensor.matmul(h_lo[:], kmn_mk[:, mk, :], wrd_r[mk][:, 0:512],
                         start=(mk == 0), stop=(mk == MT - 1),
                         skip_group_check=True)
        nc.tensor.matmul(h_hi[:], kmn_mk[:, mk, :], wrd_r[mk][:, 512:M],
                         start=(mk == 0), stop=(mk == MT - 1),
                         skip_group_check=True)
    H_r = rp.tile([128, M], F32R, tag="H_r")
    nc.vector.tensor_copy(H_r[:, 0:512], h_lo[:])
    nc.vector.tensor_copy(H_r[:, 512:M], h_hi[:])

    # ======================= PHASE 2 =====================================
    for h in range(NCH):
        qslice = qryT_r[:, h * CHUNK:(h + 1) * CHUNK]

        u_ps = ps_acc.tile([128, CHUNK], F32, tag="slot_c")
        s_ps = ps_acc.tile([1, CHUNK], F32, tag="slot_d")
        for mp in range(MT):
            prl = ps_mm.tile([128, CHUNK], F32, tag="mm")
            nc.tensor.matmul(prl[:], H_r[:, mp * 128:(mp + 1) * 128], qslice,
                             start=True, stop=True)
            erT = sp.tile([128, CHUNK], F32R, tag="erT", bufs=2)
            nc.scalar.activation(erT[:], prl[:], AF.Exp,
                                 bias=bias_rd[:, mp:mp + 1])
            nc.tensor.matmul(u_ps[:], vmn_r[:, mp, :], erT[:],
                             start=(mp == 0), stop=(mp == MT - 1),
                             skip_group_check=True)
            nc.tensor.matmul(s_ps[:], ones_r[:], erT[:],
                             start=(mp == 0), stop=(mp == MT - 1),
                             skip_group_check=True)

        # transpose denominators [1, CHUNK] -> [128, NBT] and invert
        s_sb = sp.tile([1, CHUNK], F32, tag="s_sb")
        nc.any.tensor_copy(s_sb[:], s_ps[:])
        s_cols = sp.tile([128, NBT], F32, tag="s_cols")
        for t in range(NBT):
            pst = ps_tr.tile([128, 1], F32, tag="tr")
            nc.tensor.matmul(pst[:], s_sb[0:1, t * 128:(t + 1) * 128],
                             one1[:], start=True, stop=True)
            nc.vector.tensor_copy(s_cols[:, t:t + 1], pst[:])
        r_cols = sp.tile([128, NBT], F32, tag="r_cols")
        nc.vector.reciprocal(r_cols[:], s_cols[:])

        # read_vec: transpose u back to batch-major, scale, store
        u_sb = sp.tile([128, CHUNK], F32, tag="u_sb")
        nc.any.tensor_copy(u_sb[:], u_ps[:])
        for t in range(NBT):
            ptu = ps_tr.tile([128, 128], F32, tag="tr")
            nc.tensor.matmul(ptu[:], u_sb[:, t * 128:(t + 1) * 128],
                             ident[:], is_transpose=True,
                             start=True, stop=True)
            ot = sp.tile([128, V], F32, tag="ot", bufs=3)
            nc.vector.tensor_scalar_mul(ot[:], ptu[:], r_cols[:, t:t + 1])
            r0 = h * CHUNK + t * 128
            nc.sync.dma_start(y[r0:r0 + 128, :], ot[:])


_NC_CACHE = None


def _get_nc():
    global _NC_CACHE
    if _NC_CACHE is None:
        _NC_CACHE = build_nc()
    return _NC_CACHE


def kernel(**inputs):
    nc = _get_nc()
    xs = np.ascontiguousarray(np.asarray(inputs["x"], dtype=np.float32))
    rep = {}
    for name in ("Wk", "Wv", "Wq", "Wwr", "Wrd", "key_memory", "value_memory"):
        rep[name] = np.ascontiguousarray(np.asarray(inputs[name], np.float32))
    for name in ("bk", "bv", "bq", "bwr", "brd"):
        rep[name] = np.ascontiguousarray(
            np.asarray(inputs[name], np.float32).reshape(-1, 1))
    in_maps = []
    for c in range(N_CORES):
        m = {"x": xs[c * B_LOC:(c + 1) * B_LOC]}
        m.update(rep)
        in_maps.append(m)
    res = run_bass_kernel_spmd(nc, in_maps, core_ids=list(range(N_CORES)))
    return np.concatenate([r["y"] for r in res.results], axis=0)


# revision 13
# speedup vs baseline: 1.0487x; 1.0007x over previous
"""MemoryAugmentedLayer kernel for 8 trn2 NeuronCores.

Data-parallel over batch B=32768 (4096 rows/core); the two einsum partial
sums ([M,K] and [M,V]) are all-reduced between the write and read phases.

Structure (per core):
- Associativity: write logits = key_vec @ G with G = key_memory.T @ Wwr
  precomputed once (and read logits = qry_vec @ H with H = km_new.T @ Wrd
  computed once after the all-reduce). This removes the [B,M] score
  intermediates entirely and keeps every streaming matmul a 128-deep
  contraction with the weight stationary.
- Activations stay feature-major ([feat, batch]) on chip; PE transposes
  produce the batch-major copies the einsum and the output need.
- Precision: read path in float32r (~1e-4 rel err, full PE rate); write
  path (G, exp weights, einsum operands) in bf16 — it only perturbs the
  memory update, which is a ~5% correction to the memories.
"""

import numpy as np

import concourse.bacc as bacc
import concourse.mybir as mybir
import concourse.tile as tile
from concourse import masks
from concourse.bass_utils import run_bass_kernel_spmd

F32 = mybir.dt.float32
F32R = mybir.dt.float32r
BF16 = mybir.dt.bfloat16

B, D, M, K, V = 32768, 256, 1024, 128, 128
N_CORES = 8
B_LOC = B // N_CORES          # 4096 rows per core
CHUNK = 512                   # batch columns processed per chunk
NCH = B_LOC // CHUNK          # 8 chunks
NBT = CHUNK // 128            # 4 batch tiles of 128 per chunk
MT = M // 128                 # 8 tiles of the memory dim
DT = D // 128                 # 2 tiles of the input dim
INV_B = 1.0 / B


def build_nc(repeat=1):
    nc = bacc.Bacc("TRN2", target_bir_lowering=False, debug=False,
                   num_devices=N_CORES)

    x = nc.dram_tensor("x", [B_LOC, D], F32, kind="ExternalInput")
    Wk = nc.dram_tensor("Wk", [D, K], F32, kind="ExternalInput")
    Wv = nc.dram_tensor("Wv", [D, V], F32, kind="ExternalInput")
    Wq = nc.dram_tensor("Wq", [D, K], F32, kind="ExternalInput")
    bk = nc.dram_tensor("bk", [K, 1], F32, kind="ExternalInput")
    bv = nc.dram_tensor("bv", [V, 1], F32, kind="ExternalInput")
    bq = nc.dram_tensor("bq", [K, 1], F32, kind="ExternalInput")
    Wwr = nc.dram_tensor("Wwr", [M, M], F32, kind="ExternalInput")
    Wrd = nc.dram_tensor("Wrd", [M, M], F32, kind="ExternalInput")
    bwr = nc.dram_tensor("bwr", [M, 1], F32, kind="ExternalInput")
    brd = nc.dram_tensor("brd", [M, 1], F32, kind="ExternalInput")
    km = nc.dram_tensor("key_memory", [M, K], F32, kind="ExternalInput")
    vm = nc.dram_tensor("value_memory", [M, V], F32, kind="ExternalInput")
    y = nc.dram_tensor("y", [B_LOC, V], F32, kind="ExternalOutput")

    with tile.TileContext(nc) as tc:
        _emit(nc, tc, x, Wk, Wv, Wq, bk, bv, bq, Wwr, Wrd, bwr, brd, km, vm, y,
              repeat=repeat)
    nc.compile()
    return nc


def _emit(nc, tc, x, Wk, Wv, Wq, bk, bv, bq, Wwr, Wrd, bwr, brd, km, vm, y,
          repeat=1):
    AF = mybir.ActivationFunctionType
    ALU = mybir.AluOpType

    with (
        tc.tile_pool(name="resident", bufs=1) as rp,
        tc.tile_pool(name="stage", bufs=2) as stage,
        tc.tile_pool(name="stream", bufs=2) as sp,
        tc.tile_pool(name="stream1", bufs=1) as sp1,
        tc.tile_pool(name="ps_acc", bufs=1, space="PSUM") as ps_acc,
        tc.tile_pool(name="ps_mm", bufs=2, space="PSUM") as ps_mm,
        tc.tile_pool(name="ps_tr", bufs=1, space="PSUM") as ps_tr,
        tc.tile_pool(name="dram", bufs=1, space="DRAM") as dp,
    ):
        # ---------------- setup: identities, ones, biases ----------------
        ident = rp.tile([128, 128], F32)
        masks.make_identity(nc, ident[:])
        ident_b = rp.tile([128, 128], BF16)
        nc.vector.tensor_copy(ident_b[:], ident[:])

        ones_f = rp.tile([128, 1], F32)
        nc.gpsimd.memset(ones_f[:], 1.0)
        ones_r = rp.tile([128, 1], F32R)
        nc.vector.tensor_copy(ones_r[:], ones_f[:])
        one1 = rp.tile([1, 1], F32)
        nc.gpsimd.memset(one1[:], 1.0)

        # projection weights as lhsT ([d,128] blocks), rounded to f32r
        projw_r = rp.tile([128, DT, 3, 128], F32R)
        for j, W in enumerate((Wk, Wv, Wq)):
            for dt in range(DT):
                wst = stage.tile([128, 128], F32, tag="wst")
                nc.sync.dma_start(wst[:], W[dt * 128:(dt + 1) * 128, :])
                nc.vector.tensor_copy(projw_r[:, dt, j, :], wst[:])

        bias_p = rp.tile([128, 3], F32)
        for j, b in enumerate((bk, bv, bq)):
            nc.sync.dma_start(bias_p[:, j:j + 1], b[:])
        bias_wr = rp.tile([128, MT], F32)
        bias_rd = rp.tile([128, MT], F32)
        for mp in range(MT):
            nc.sync.dma_start(bias_wr[:, mp:mp + 1], bwr[mp * 128:(mp + 1) * 128, :])
            nc.sync.dma_start(bias_rd[:, mp:mp + 1], brd[mp * 128:(mp + 1) * 128, :])

        # ---- G = key_memory.T @ Wwr (bf16), kmT/vmT (transposed, f32) ----
        kmT_f = rp.tile([128, M], F32)
        vmT_f = rp.tile([128, M], F32)
        g_lo = ps_acc.tile([128, 512], F32, tag="slot_a")
        g_hi = ps_acc.tile([128, 512], F32, tag="slot_b")
        for mk in range(MT):
            mst = stage.tile([128, 128], F32, tag="mst")
            nc.sync.dma_start(mst[:], km[mk * 128:(mk + 1) * 128, :])
            km_b = stage.tile([128, 128], BF16, tag="km_b")
            nc.vector.tensor_copy(km_b[:], mst[:])
            wwrt = stage.tile([128, M], F32, tag="wbig")
            nc.sync.dma_start(wwrt[:], Wwr[mk * 128:(mk + 1) * 128, :])
            wwrt_b = stage.tile([128, M], BF16, tag="wbig_b")
            nc.vector.tensor_copy(wwrt_b[:], wwrt[:])
            nc.tensor.matmul(g_lo[:], km_b[:], wwrt_b[:, 0:512],
                             start=(mk == 0), stop=(mk == MT - 1),
                             skip_group_check=True)
            nc.tensor.matmul(g_hi[:], km_b[:], wwrt_b[:, 512:M],
                             start=(mk == 0), stop=(mk == MT - 1),
                             skip_group_check=True)
            ptr = ps_tr.tile([128, 128], F32, tag="tr")
            nc.tensor.matmul(ptr[:], mst[:], ident[:], is_transpose=True,
                             start=True, stop=True)
            nc.scalar.copy(kmT_f[:, mk * 128:(mk + 1) * 128], ptr[:])
            mst2 = stage.tile([128, 128], F32, tag="mst")
            nc.sync.dma_start(mst2[:], vm[mk * 128:(mk + 1) * 128, :])
            ptr2 = ps_tr.tile([128, 128], F32, tag="tr")
            nc.tensor.matmul(ptr2[:], mst2[:], ident[:], is_transpose=True,
                             start=True, stop=True)
            nc.scalar.copy(vmT_f[:, mk * 128:(mk + 1) * 128], ptr2[:])
        G_b = rp.tile([128, M], BF16)
        nc.vector.tensor_copy(G_b[:, 0:512], g_lo[:])
        nc.vector.tensor_copy(G_b[:, 512:M], g_hi[:])

        # Wrd resident as lhsT tiles [128, M] f32r (read path)
        wrd_r = [rp.tile([128, M], F32R, name=f"wrd_r{i}") for i in range(MT)]
        for mk in range(MT):
            wst3 = stage.tile([128, M], F32, tag="wbig")
            nc.sync.dma_start(wst3[:], Wrd[mk * 128:(mk + 1) * 128, :])
            nc.vector.tensor_copy(wrd_r[mk][:], wst3[:])

        # qry kept for phase 2
        qryT_r = rp.tile([128, B_LOC], F32R)

        for _rep in range(repeat):
            _emit_rep(nc, tc, x, y, rp, sp, sp1, ps_acc, ps_mm, ps_tr, dp,
                      ident, ident_b, ones_r, one1, projw_r, bias_p, bias_wr,
                      bias_rd, G_b, wrd_r, kmT_f, vmT_f, qryT_r)


def _emit_rep(nc, tc, x, y, rp, sp, sp1, ps_acc, ps_mm, ps_tr, dp,
              ident, ident_b, ones_r, one1, projw_r, bias_p, bias_wr,
              bias_rd, G_b, wrd_r, kmT_f, vmT_f, qryT_r):
    AF = mybir.ActivationFunctionType
    ALU = mybir.AluOpType
    AX = mybir.AxisListType

    # einsum partial accumulators, PSUM-resident across phase 1
    pk_lo = ps_acc.tile([128, 512], F32, tag="slot_a")
    pk_hi = ps_acc.tile([128, 512], F32, tag="slot_b")
    pv_lo = ps_acc.tile([128, 512], F32, tag="slot_c")
    pv_hi = ps_acc.tile([128, 512], F32, tag="slot_d")

    # ======================= PHASE 1 =====================================
    for h in range(NCH):
        # ---- load + transpose x chunk -> xTr [128, dtile, CHUNK] f32r
        xTr = sp.tile([128, DT, CHUNK], F32R, tag="xTr")
        for t in range(NBT):
            r0 = h * CHUNK + t * 128
            xa = sp.tile([128, D], F32, tag="xa", bufs=3)
            nc.sync.dma_start(xa[:], x[r0:r0 + 128, :])
            for dt in range(DT):
                ptx = ps_tr.tile([128, 128], F32, tag="tr")
                nc.tensor.matmul(ptx[:], xa[:, dt * 128:(dt + 1) * 128],
                                 ident[:], is_transpose=True,
                                 start=True, stop=True)
                nc.any.tensor_copy(xTr[:, dt, t * 128:(t + 1) * 128], ptx[:])

        # ---- projections + elu -> kvT/vvT (bf16), qryT (f32r)
        kvT = sp.tile([128, CHUNK], BF16, tag="kvT")
        vvT = sp.tile([128, CHUNK], BF16, tag="vvT")
        for j in range(3):
            pp = ps_mm.tile([128, CHUNK], F32, tag="mm")
            for dt in range(DT):
                nc.tensor.matmul(pp[:], projw_r[:, dt, j, :], xTr[:, dt, :],
                                 start=(dt == 0), stop=(dt == DT - 1))
            bcol = bias_p[:, j:j + 1]
            # elu(z+b) = relu(z+b) + exp(min(z+b,0)) - 1
            tmin = sp.tile([128, CHUNK], F32, tag="tmin", bufs=2)
            nc.vector.tensor_scalar(out=tmin[:], in0=pp[:], scalar1=bcol,
                                    scalar2=0.0, op0=ALU.add, op1=ALU.min)
            texp = sp.tile([128, CHUNK], F32, tag="texp", bufs=2)
            nc.scalar.activation(texp[:], tmin[:], AF.Exp)
            trelu = sp.tile([128, CHUNK], F32, tag="trelu", bufs=2)
            nc.vector.tensor_scalar(out=trelu[:], in0=pp[:], scalar1=bcol,
                                    scalar2=0.0, op0=ALU.add, op1=ALU.max)
            dst = (kvT[:], vvT[:],
                   qryT_r[:, h * CHUNK:(h + 1) * CHUNK])[j]
            nc.vector.scalar_tensor_tensor(dst, texp[:], -1.0, trelu[:],
                                           ALU.add, ALU.add)

        # ---- write logits (via G) + exp + batched transpose to batch-major
        expw_bm = sp1.tile([128, NBT, M], BF16, tag="expw_bm", bufs=2)
        for mp in range(MT):
            pwl = ps_mm.tile([128, CHUNK], F32, tag="mm")
            nc.tensor.matmul(pwl[:], G_b[:, mp * 128:(mp + 1) * 128], kvT[:],
                             start=True, stop=True)
            eT = sp.tile([128, CHUNK], BF16, tag="eT", bufs=2)
            nc.scalar.activation(eT[:], pwl[:], AF.Exp,
                                 bias=bias_wr[:, mp:mp + 1])
            ptb = ps_tr.tile([128, NBT, 128], BF16, tag="trb")
            for t in range(NBT):
                nc.tensor.matmul(ptb[:, t, :], eT[:, t * 128:(t + 1) * 128],
                                 ident_b[:], is_transpose=True,
                                 start=True, stop=True, skip_group_check=True)
            nc.any.tensor_copy(expw_bm[:, :, mp * 128:(mp + 1) * 128], ptb[:])

        # ---- softmax denominators (per batch row) + scaled kv/vv (bm)
        rw = sp.tile([128, NBT], F32, tag="rw")
        sw = sp.tile([128, NBT], F32, tag="sw")
        for t in range(NBT):
            nc.vector.tensor_reduce(sw[:, t:t + 1], expw_bm[:, t, :],
                                    AX.X, ALU.add)
        nc.vector.reciprocal(rw[:], sw[:])

        kv_sc = sp.tile([128, NBT, 128], BF16, tag="kv_sc")
        vv_sc = sp.tile([128, NBT, 128], BF16, tag="vv_sc")
        for src, dstt in ((kvT, kv_sc), (vvT, vv_sc)):
            ptk = ps_tr.tile([128, NBT, 128], BF16, tag="trb")
            for t in range(NBT):
                nc.tensor.matmul(ptk[:, t, :], src[:, t * 128:(t + 1) * 128],
                                 ident_b[:], is_transpose=True,
                                 start=True, stop=True, skip_group_check=True)
            for t in range(NBT):
                nc.vector.tensor_scalar_mul(dstt[:, t, :], ptk[:, t, :],
                                            rw[:, t:t + 1])

        # ---- einsum partials, accumulated in PSUM across all chunks
        for t in range(NBT):
            f = h == 0 and t == 0
            l = h == NCH - 1 and t == NBT - 1
            nc.tensor.matmul(pk_lo[:], kv_sc[:, t, :], expw_bm[:, t, 0:512],
                             start=f, stop=l, skip_group_check=True)
            nc.tensor.matmul(pk_hi[:], kv_sc[:, t, :], expw_bm[:, t, 512:M],
                             start=f, stop=l, skip_group_check=True)
            nc.tensor.matmul(pv_lo[:], vv_sc[:, t, :], expw_bm[:, t, 0:512],
                             start=f, stop=l, skip_group_check=True)
            nc.tensor.matmul(pv_hi[:], vv_sc[:, t, :], expw_bm[:, t, 512:M],
                             start=f, stop=l, skip_group_check=True)

    # ================== ALLREDUCE of partials ============================
    part_sb = rp.tile([128, 2 * M], F32, tag="part_sb")
    nc.vector.tensor_copy(part_sb[:, 0:512], pk_lo[:])
    nc.vector.tensor_copy(part_sb[:, 512:1024], pk_hi[:])
    nc.vector.tensor_copy(part_sb[:, 1024:1536], pv_lo[:])
    nc.vector.tensor_copy(part_sb[:, 1536:2048], pv_hi[:])
    cc_in = dp.tile([128, 2 * M], F32, tag="cc_in")
    cc_out = dp.tile([128, 2 * M], F32, tag="cc_out")
    nc.sync.dma_start(cc_in[:], part_sb[:])
    nc.gpsimd.collective_compute(
        "AllReduce", mybir.AluOpType.add,
        replica_groups=[list(range(N_CORES))],
        ins=[cc_in.opt()], outs=[cc_out.opt()],
    )
    red_sb = rp.tile([128, 2 * M], F32, tag="red_sb")
    nc.sync.dma_start(red_sb[:], cc_out[:])

    # ---- memory update + H = km_new.T @ Wrd (f32r) ----------------------
    km_newT = rp.tile([128, M], F32, tag="km_newT")
    nc.vector.scalar_tensor_tensor(km_newT[:], red_sb[:, 0:M], INV_B,
                                   kmT_f[:], ALU.mult, ALU.add)
    vm_newT = rp.tile([128, M], F32, tag="vm_newT")
    nc.vector.scalar_tensor_tensor(vm_newT[:], red_sb[:, M:2 * M], INV_B,
                                   vmT_f[:], ALU.mult, ALU.add)
    # transpose km_new/vm_new back to [m, *] blocks (f32r)
    kmn_mk = rp.tile([128, MT, 128], F32R, tag="kmn_mk")
    vmn_r = rp.tile([128, MT, 128], F32R, tag="vmn_r")
    for mk in range(MT):
        ptm = ps_tr.tile([128, 128], F32, tag="tr")
        nc.tensor.matmul(ptm[:], km_newT[:, mk * 128:(mk + 1) * 128],
                         ident[:], is_transpose=True, start=True, stop=True)
        nc.any.tensor_copy(kmn_mk[:, mk, :], ptm[:])
        ptm2 = ps_tr.tile([128, 128], F32, tag="tr")
        nc.tensor.matmul(ptm2[:], vm_newT[:, mk * 128:(mk + 1) * 128],
                         ident[:], is_transpose=True, start=True, stop=True)
        nc.any.tensor_copy(vmn_r[:, mk, :], ptm2[:])
    h_lo = ps_acc.tile([128, 512], F32, tag="slot_a")
    h_hi = ps_acc.tile([128, 512], F32, tag="slot_b")
    for mk in range(MT):
        nc.tensor.matmul(h_lo[:], kmn_mk[:, mk, :], wrd_r[mk][:, 0:512],
                         start=(mk == 0), stop=(mk == MT - 1),
                         skip_group_check=True)
        nc.tensor.matmul(h_hi[:], kmn_mk[:, mk, :], wrd_r[mk][:, 512:M],
                         start=(mk == 0), stop=(mk == MT - 1),
                         skip_group_check=True)
    H_r = rp.tile([128, M], F32R, tag="H_r")
    nc.vector.tensor_copy(H_r[:, 0:512], h_lo[:])
    nc.vector.tensor_copy(H_r[:, 512:M], h_hi[:])

    # ======================= PHASE 2 =====================================
    for h in range(NCH):
        qslice = qryT_r[:, h * CHUNK:(h + 1) * CHUNK]

        u_ps = ps_acc.tile([128, CHUNK], F32, tag="slot_c")
        s_ps = ps_acc.tile([1, CHUNK], F32, tag="slot_d")
        for mp in range(MT):
            prl = ps_mm.tile([128, CHUNK], F32, tag="mm")
            nc.tensor.matmul(prl[:], H_r[:, mp * 128:(mp + 1) * 128], qslice,
                             start=True, stop=True)
            erT = sp.tile([128, CHUNK], F32R, tag="erT", bufs=2)
            nc.scalar.activation(erT[:], prl[:], AF.Exp,
                                 bias=bias_rd[:, mp:mp + 1])
            nc.tensor.matmul(u_ps[:], vmn_r[:, mp, :], erT[:],
                             start=(mp == 0), stop=(mp == MT - 1),
                             skip_group_check=True)
            nc.tensor.matmul(s_ps[:], ones_r[:], erT[:],
                             start=(mp == 0), stop=(mp == MT - 1),
                             skip_group_check=True)

        # transpose denominators [1, CHUNK] -> [128, NBT] and invert
        s_sb = sp.tile([1, CHUNK], F32, tag="s_sb")
        nc.any.tensor_copy(s_sb[:], s_ps[:])
        s_cols = sp.tile([128, NBT], F32, tag="s_cols")
        for t in range(NBT):
            pst = ps_tr.tile([128, 1], F32, tag="tr")
            nc.tensor.matmul(pst[:], s_sb[0:1, t * 128:(t + 1) * 128],
                             one1[:], start=True, stop=True)
            nc.vector.tensor_copy(s_cols[:, t:t + 1], pst[:])
        r_cols = sp.tile([128, NBT], F32, tag="r_cols")
        nc.vector.reciprocal(r_cols[:], s_cols[:])

        # read_vec: transpose u back to batch-major, scale, store
        u_sb = sp.tile([128, CHUNK], F32, tag="u_sb")
        nc.any.tensor_copy(u_sb[:], u_ps[:])
        for t in range(NBT):
            ptu = ps_tr.tile([128, 128], F32, tag="tr")
            nc.tensor.matmul(ptu[:], u_sb[:, t * 128:(t + 1) * 128],
                             ident[:], is_transpose=True,
                             start=True, stop=True)
            ot = sp.tile([128, V], F32, tag="ot", bufs=3)
            nc.vector.tensor_scalar_mul(ot[:], ptu[:], r_cols[:, t:t + 1])
            r0 = h * CHUNK + t * 128
            nc.sync.dma_start(y[r0:r0 + 128, :], ot[:])


_NC_CACHE = None


def _get_nc():
    global _NC_CACHE
    if _NC_CACHE is None:
        _NC_CACHE = build_nc()
    return _NC_CACHE


def kernel(**inputs):
    nc = _get_nc()
    xs = np.ascontiguousarray(np.asarray(inputs["x"], dtype=np.float32))
    rep = {}
    for name in ("Wk", "Wv", "Wq", "Wwr", "Wrd", "key_memory", "value_memory"):
        rep[name] = np.ascontiguousarray(np.asarray(inputs[name], np.float32))
    for name in ("bk", "bv", "bq", "bwr", "brd"):
        rep[name] = np.ascontiguousarray(
            np.asarray(inputs[name], np.float32).reshape(-1, 1))
    in_maps = []
    for c in range(N_CORES):
        m = {"x": xs[c * B_LOC:(c + 1) * B_LOC]}
        m.update(rep)
        in_maps.append(m)
    res = run_bass_kernel_spmd(nc, in_maps, core_ids=list(range(N_CORES)))
    return np.concatenate([r["y"] for r in res.results], axis=0)
